# revision 30
# baseline (speedup 1.0000x reference)
"""Trainium2 8-core tensor-parallel attention kernel (Bass/Tile).

nn_Attention_5557687681160: B=2, S=1024, DIM=4096, H=32, KVH=8, HD=128, RANK=8
Sharding: tensor-parallel over heads (4 q heads + 1 kv head per core),
row-parallel wo with chunked bf16 ReduceScatter over the token axis.

LoRA adapters are folded into effective weights on the host:
    x @ w.T + (x @ a.T) @ b.T == x @ (w + b @ a).T
The 1/sqrt(HD) score scale is folded into wq. Q/K channels are permuted
per-head to [even, odd] so RoPE pairs become contiguous partition halves.
"""

import sys
import numpy as np

for _p in ("/opt/trn_rl_repo",):
    if _p not in sys.path:
        sys.path.insert(0, _p)

import ml_dtypes

BF16 = ml_dtypes.bfloat16

B, S, DIM, H, KVH, HD, RANK = 2, 1024, 4096, 32, 8, 128, 8
NCORES = 8
T = B * S                  # 2048 tokens total
QH = H // NCORES           # 4 q heads per core
QD = QH * HD               # 512 q channels per core
NB_D = DIM // 128          # 32 contraction tiles
N_TH = T // 512            # 4 token halves of 512
N_TT = T // 128            # 16 token tiles of 128
# ReduceScatter chunks: (first m-tile, #m-tiles); uneven so the last RS
# per batch is small (shorter exposed tail). m-tile = 128 tokens.
CHUNK_MT = [(0, 4), (4, 4), (8, 4), (12, 2), (14, 2)]
CHUNKS = len(CHUNK_MT)
CH_ROWS = [n * 128 // NCORES for _, n in CHUNK_MT]      # per-core rows
CH_OFF = [sum(CH_ROWS[:i]) for i in range(CHUNKS)]       # rs_out row offsets

_CACHE = {}


def _build(use_mask: bool):
    from concourse import bass, bacc, tile, mybir
    from concourse.masks import make_identity
    from contextlib import ExitStack

    f32 = mybir.dt.float32
    bf16 = mybir.dt.bfloat16
    Exp = mybir.ActivationFunctionType.Exp

    nc = bacc.Bacc(
        "TRN2", target_bir_lowering=False, debug=False, num_devices=NCORES
    )

    xT_e = nc.dram_tensor("xT", [2, NB_D, 128, 1024], bf16, kind="ExternalInput")
    wqA_e = nc.dram_tensor("wqkvA", [NB_D, 128, QD], bf16, kind="ExternalInput")
    wqB_e = nc.dram_tensor("wqkvB", [NB_D, 128, 2 * HD], bf16, kind="ExternalInput")
    woT_e = nc.dram_tensor("woT", [QD, DIM], bf16, kind="ExternalInput")
    cs1_e = nc.dram_tensor("cs1", [HD, T], bf16, kind="ExternalInput")
    cs2_e = nc.dram_tensor("cs2", [HD, T], bf16, kind="ExternalInput")
    if use_mask:
        mask_e = nc.dram_tensor("maskT", [S, S], bf16, kind="ExternalInput")
    out_e = nc.dram_tensor("out", [T // NCORES, DIM], bf16, kind="ExternalOutput")

    with tile.TileContext(nc) as tc, ExitStack() as ctx:
        const = ctx.enter_context(tc.tile_pool(name="const", bufs=1))
        persist = ctx.enter_context(tc.tile_pool(name="persist", bufs=1))
        raw = ctx.enter_context(tc.tile_pool(name="raw", bufs=1))
        xpool = ctx.enter_context(tc.tile_pool(name="xpool", bufs=6))
        wpool = ctx.enter_context(tc.tile_pool(name="wpool", bufs=6))
        ptpool = ctx.enter_context(tc.tile_pool(name="ptpool", bufs=44 if not use_mask else 34))
        rpool = ctx.enter_context(tc.tile_pool(name="rpool", bufs=2))
        stpool = ctx.enter_context(tc.tile_pool(name="stpool", bufs=3 if not use_mask else 2))
        ps = ctx.enter_context(
            tc.tile_pool(name="ps", bufs=4, space=bass.MemorySpace.PSUM)
        )
        pswo = ctx.enter_context(
            tc.tile_pool(name="pswo", bufs=4, space=bass.MemorySpace.PSUM)
        )
        dram = ctx.enter_context(
            tc.tile_pool(name="dram", bufs=1, space="DRAM")
        )

        # ---- constants / persistent tensors ----
        ident = const.tile([128, 128], bf16, tag="ident")
        make_identity(nc, ident[:])
        ones = const.tile([128, 1], bf16, tag="ones")
        nc.gpsimd.memset(ones[:], 1.0)

        cs1_sb = persist.tile([HD, T], bf16, tag="cs1")
        cs2_sb = persist.tile([HD, T], bf16, tag="cs2")
        wo_sb = [persist.tile([128, DIM], bf16, tag=f"wo{i}", name=f"wo{i}") for i in range(4)]
        if use_mask:
            mask_sb = [
                persist.tile([128, S], bf16, tag=f"mk{i}", name=f"mk{i}") for i in range(8)
            ]

        # raw (pre-RoPE) channel-major projections: q0..q3, k, v
        qk_raw = [raw.tile([128, T], bf16, tag=f"raw{c}", name=f"raw{c}") for c in range(6)]
        # post-RoPE
        qtr = [persist.tile([128, T], bf16, tag=f"qtr{c}", name=f"qtr{c}") for c in range(4)]
        ktr = persist.tile([128, T], bf16, tag="ktr")
        # token-major V tiles
        vtok = [persist.tile([128, 128], bf16, tag=f"vt{t}", name=f"vt{t}") for t in range(N_TT)]
        # attention output (channel-major, per local qd tile)
        aout = [persist.tile([128, T], bf16, tag=f"ao{c}", name=f"ao{c}") for c in range(4)]

        # ---- phase 1: fused QKV projection (channel-major) ----
        # Two sweeps over x^T: A = q heads (4 ch), B = k+v (2 ch).
        # Each stationary weight tile feeds 2 moving matmuls (1024 tokens).
        for sweep, (w_e, chs) in enumerate([(wqA_e, range(4)), (wqB_e, range(4, 6))]):
            nch = len(chs)
            for tq in range(2):
                psq = [[(ps if (ci * 2 + j) % 2 == 0 else pswo).tile(
                            [128, 512], f32,
                            tag="mm" if (ci * 2 + j) % 2 == 0 else "wo",
                            name="psq")
                        for j in range(2)] for ci in range(nch)]
                for d in range(NB_D):
                    xt = xpool.tile([128, 1024], bf16, tag="xt")
                    nc.sync.dma_start(xt[:], xT_e[tq, d])
                    wt = wpool.tile([128, 128 * nch], bf16, tag="wt")
                    nc.sync.dma_start(wt[:], w_e[d])
                    for ci in range(nch):
                        for j in range(2):
                            nc.tensor.matmul(
                                psq[ci][j][:],
                                wt[:, ci * 128 : (ci + 1) * 128],
                                xt[:, j * 512 : (j + 1) * 512],
                                start=(d == 0),
                                stop=(d == NB_D - 1),
                            )
                for ci, c in enumerate(chs):
                    for j in range(2):
                        nc.scalar.copy(
                            qk_raw[c][:, tq * 1024 + j * 512 : tq * 1024 + (j + 1) * 512],
                            psq[ci][j][:],
                        )

        # ---- phase 2: V transpose to token-major ----
        for t in range(N_TT):
            pt_ps = ps.tile([128, 128], bf16, tag="mm")
            nc.tensor.transpose(
                pt_ps[:], qk_raw[5][:, t * 128 : (t + 1) * 128], ident[:]
            )
            nc.scalar.copy(vtok[t][:], pt_ps[:])

        # persistent loads deferred here so phase-1 DMA gets the bus first
        nc.sync.dma_start(cs1_sb[:], cs1_e[:])
        nc.sync.dma_start(cs2_sb[:], cs2_e[:])
        for i in range(4):
            nc.sync.dma_start(wo_sb[i][:], woT_e[i * 128 : (i + 1) * 128, :])
        if use_mask:
            for i in range(8):
                nc.sync.dma_start(mask_sb[i][:], mask_e[i * 128 : (i + 1) * 128, :])

        # ---- phase 3: RoPE on q0..q3 and k ----
        # channel layout per head tile: [64 even pairs; 64 odd pairs]
        def rope(dst, src):
            # out = src * [cos;cos] + [odd;even] * [-sin;sin]
            rv = rpool.tile([128, T], bf16, tag="rv", name="rv", bufs=1)
            nc.vector.tensor_copy(rv[0:64, :], src[64:128, :])
            nc.vector.tensor_copy(rv[64:128, :], src[0:64, :])
            tmp = rpool.tile([128, T], bf16, tag="rtmp", name="rtmp", bufs=1)
            nc.vector.tensor_mul(dst[:], src[:], cs1_sb[:])
            nc.vector.tensor_mul(tmp[:], rv[:], cs2_sb[:])
            nc.vector.tensor_add(dst[:], dst[:], tmp[:])

        rope(ktr, qk_raw[4])
        for c in range(4):
            rope(qtr[c], qk_raw[c])

        # ---- phase 4+5: per batch: attention, then wo + ReduceScatter ----
        partial = dram.tile([T, DIM], bf16, tag="partial")
        rs_out = dram.tile([T // NCORES, DIM], bf16, tag="rsout")

        def wo_chunk(mc):
            m0, nm = CHUNK_MT[mc]
            for m in range(m0, m0 + nm):
                st = stpool.tile([128, DIM], bf16, tag="st")
                # stationary aout[c][m] reused across 4 moving n-tiles
                for nh in range(2):
                    wp = [pswo.tile([128, 512], f32, tag="wo", name="wp")
                          for _ in range(4)]
                    for c in range(4):
                        for n in range(4):
                            nc.tensor.matmul(
                                wp[n][:],
                                aout[c][:, m * 128 : (m + 1) * 128],
                                wo_sb[c][:, (nh * 4 + n) * 512 : (nh * 4 + n + 1) * 512],
                                start=(c == 0),
                                stop=(c == 3),
                            )
                    for n in range(4):
                        nc.scalar.copy(
                            st[:, (nh * 4 + n) * 512 : (nh * 4 + n + 1) * 512],
                            wp[n][:],
                        )
                nc.sync.dma_start(partial[m * 128 : (m + 1) * 128, :], st[:])
            r0, nr = CH_OFF[mc], CH_ROWS[mc]
            nc.gpsimd.collective_compute(
                "ReduceScatter",
                mybir.AluOpType.add,
                replica_groups=[list(range(NCORES))],
                ins=[partial[m0 * 128 : (m0 + nm) * 128, :].opt()],
                outs=[rs_out[r0 : r0 + nr, :].opt()],
            )
            nc.sync.dma_start(out_e[r0 : r0 + nr, :], rs_out[r0 : r0 + nr, :])

        def attention_scores(b, hq, sh):
            base = b * S + sh * 512
            pt = [ptpool.tile([128, 512], bf16, tag="pt", name="pt")
                  for _ in range(8)]
            for ti in range(8):
                sc = ps.tile([128, 512], f32, tag="mm", name="sc")
                nc.tensor.matmul(
                    sc[:],
                    ktr[:, b * S + ti * 128 : b * S + (ti + 1) * 128],
                    qtr[hq][:, base : base + 512],
                    start=True,
                    stop=True,
                )
                if use_mask:
                    tmp = ptpool.tile([128, 512], bf16, tag="pt", name="sctmp")
                    nc.vector.tensor_add(
                        tmp[:], sc[:], mask_sb[ti][:, sh * 512 : (sh + 1) * 512]
                    )
                    nc.scalar.activation(pt[ti][:], tmp[:], Exp)
                else:
                    nc.scalar.activation(pt[ti][:], sc[:], Exp)
            return pt

        def attention_pv(b, hq, sh, pt):
            base = b * S + sh * 512
            sm = ps.tile([1, 512], f32, tag="mm", name="sm")
            for ti in range(8):
                nc.tensor.matmul(
                    sm[:], ones[:], pt[ti][:], start=(ti == 0), stop=(ti == 7)
                )
            rs_ = rpool.tile([1, 512], f32, tag="rsum", name="rs_")
            nc.vector.reciprocal(rs_[:], sm[:])
            rb = rpool.tile([128, 512], f32, tag="rb", name="rb", bufs=2)
            nc.gpsimd.partition_broadcast(rb[:], rs_[:])
            ov = ps.tile([128, 512], f32, tag="mm", name="ov")
            for ti in range(8):
                nc.tensor.matmul(
                    ov[:], vtok[b * 8 + ti][:], pt[ti][:],
                    start=(ti == 0), stop=(ti == 7),
                )
            nc.vector.tensor_mul(aout[hq][:, base : base + 512], ov[:], rb[:])

        half_chunks = {}
        for mc, (m0, nm) in enumerate(CHUNK_MT):
            half_chunks.setdefault(m0 * 128 // 512, []).append(mc)
        for b in range(B):
            for sh in range(2):
                pts = [attention_scores(b, hq, sh) for hq in range(QH)]
                for hq in range(QH):
                    attention_pv(b, hq, sh, pts[hq])
                for mc in half_chunks[b * 2 + sh]:
                    wo_chunk(mc)

    nc.compile()
    return nc


def _prep(x, freqs_cos, freqs_sin, mask, wq, wk, wv, wo,
          lq_a, lq_b, lk_a, lk_b, lv_a, lv_b, lo_a, lo_b):
    f32 = np.float32
    asf = lambda a: np.asarray(a, dtype=f32)
    x, wq, wk, wv, wo = map(asf, (x, wq, wk, wv, wo))
    lq_a, lq_b, lk_a, lk_b = map(asf, (lq_a, lq_b, lk_a, lk_b))
    lv_a, lv_b, lo_a, lo_b = map(asf, (lv_a, lv_b, lo_a, lo_b))
    mask = asf(mask)
    freqs_cos, freqs_sin = asf(freqs_cos), asf(freqs_sin)

    wq_eff = (wq + lq_b @ lq_a) * f32(1.0 / np.sqrt(HD))
    wk_eff = wk + lk_b @ lk_a
    wv_eff = wv + lv_b @ lv_a
    wo_eff = wo + lo_b @ lo_a

    # per-head channel permutation: [0,2,4,...,126, 1,3,...,127]
    perm = np.concatenate([np.arange(0, HD, 2), np.arange(1, HD, 2)])
    wq_p = wq_eff.reshape(H, HD, DIM)[:, perm, :].reshape(H * HD, DIM)
    wk_p = wk_eff.reshape(KVH, HD, DIM)[:, perm, :].reshape(KVH * HD, DIM)

    xT = x.reshape(T, DIM).T.astype(BF16)
    xT = np.ascontiguousarray(
        xT.reshape(NB_D, 128, 2, 1024).transpose(2, 0, 1, 3)
    )
    cosT = np.tile(freqs_cos.T, (1, B))
    sinT = np.tile(freqs_sin.T, (1, B))
    cs1 = np.ascontiguousarray(np.vstack([cosT, cosT])).astype(BF16)
    cs2 = np.ascontiguousarray(np.vstack([-sinT, sinT])).astype(BF16)
    use_mask = bool(np.any(mask))
    maskT = np.ascontiguousarray(mask[0, 0].T).astype(BF16) if use_mask else None

    in_maps = []
    for g in range(NCORES):
        wqT = wq_p[g * QD : (g + 1) * QD, :].T          # [DIM, 512]
        wkT = wk_p[g * HD : (g + 1) * HD, :].T          # [DIM, 128]
        wvT = wv_eff[g * HD : (g + 1) * HD, :].T        # [DIM, 128]
        wqkvA = np.ascontiguousarray(wqT).astype(BF16).reshape(NB_D, 128, QD)
        wqkvB = np.ascontiguousarray(
            np.concatenate([wkT, wvT], axis=1)
        ).astype(BF16).reshape(NB_D, 128, 2 * HD)
        woT = np.ascontiguousarray(
            wo_eff[:, g * QD : (g + 1) * QD].T
        ).astype(BF16)
        m = {"xT": xT, "wqkvA": wqkvA, "wqkvB": wqkvB, "woT": woT, "cs1": cs1, "cs2": cs2}
        if use_mask:
            m["maskT"] = maskT
        in_maps.append(m)
    return in_maps, use_mask


def _get_nc(use_mask):
    key = ("nc", use_mask)
    if key not in _CACHE:
        _CACHE[key] = _build(use_mask)
    return _CACHE[key]


def _patch_walrus():
    from concourse import bass_utils as bu
    if getattr(bu, "_ldw_patched", False):
        return
    orig = bu.run_command
    def patched(argv, **kw):
        return orig(argv, **kw)
    bu.run_command = patched
    bu._ldw_patched = True


def run(in_maps, use_mask, trace=False, **kw):
    from concourse.bass_utils import run_bass_kernel_spmd
    _patch_walrus()

    nc = _get_nc(use_mask)
    return run_bass_kernel_spmd(
        nc, in_maps, core_ids=list(range(NCORES)), trace=trace, **kw
    )


def kernel(**inputs):
    in_maps, use_mask = _prep(**inputs)
    res = run(in_maps, use_mask)
    return gather([res.results[g]["out"] for g in range(NCORES)])


def gather(core_outs):
    out = np.empty((T, DIM), np.float32)
    for g in range(NCORES):
        r = np.asarray(core_outs[g], dtype=np.float32).reshape(T // NCORES, DIM)
        for mc in range(CHUNKS):
            m0, nm = CHUNK_MT[mc]
            nr = CH_ROWS[mc]
            t0 = m0 * 128 + g * nr
            out[t0 : t0 + nr] = r[CH_OFF[mc] : CH_OFF[mc] + nr]
    return out.reshape(B, S, DIM)
